# revision 1
# baseline (speedup 1.0000x reference)
"""Trainium2 Bass kernel for nn_Attribution (sparse local-window attention).

Data-parallel over batch n=8 -> one batch element per NeuronCore.

Per-core computation (c_in=256, ch=128, 64x64 image):
    h    = W1 @ x + b1
    corr = 5x5 local window correlation of h (zero padded), /sqrt(128)
    attn = softmax over the 25 window entries
    samp = sum_k attn_k * shift_k(h)
    gate = sigmoid(relu(W2 @ h + b2)) = 0.5 + 0.5*relu(tanh((z+b2)/2))
    out  = Wout @ (gate * samp) + bout

Layout: positions flattened row-major with 2 zero-pad rows top/bottom
(68 rows x 64 = 4352 positions = 34 chunks of 128).  Scores "born
transposed" (keys of chunk c on partitions, queries on free axis).
Out-of-window entries killed by a {0,1} mask after exp; out-of-image x
neighbors accounted by denominator correction D (exp(0)=1 each in the
zero-padded reference).

This version is built around keeping the PE continuously streaming (TRN2
p-states: PE only reaches 2.4 GHz after ~3us of gap-free execution):
  - denominator colsums and sample matmuls are accumulated into rotating
    full-bank PSUM rings, one wide matmul per (chunk, psum-bank) part
    instead of 3 tiny matmuls per sub (frontier-split so each MM is
    either pure-accumulate or pure-first-write within the lazily-zeroed
    2KB PSUM zero region),
  - bias matmuls are gone (tensor bias on ACT evacuation),
  - evacuations are spread across ACT / DVE / GPSIMD,
  - reciprocal is one fused DVE op (reciprocal_approx_fast),
  - constants ride in one blob DMA; output is stored bf16 and upcast on
    host (rel-err budget 2e-2 >> bf16 rounding).
"""
import sys

sys.path.insert(0, "/opt/trn_rl_repo")

import numpy as np
import ml_dtypes

import concourse.bass as bass
import concourse.mybir as mybir
import concourse.tile as tile
from concourse import bacc
from concourse.bass_utils import run_bass_kernel_spmd

F32 = mybir.dt.float32
BF16 = mybir.dt.bfloat16
AF = mybir.ActivationFunctionType
ALU = mybir.AluOpType

N, CIN, CH, H, W = 8, 256, 128, 64, 64
HW = H * W                      # 4096
RAD = 2
KROWS = H + 2 * RAD             # 68 padded rows
PADPOS = KROWS * W              # 4352
NCHUNK = PADPOS // 128          # 34 key chunks (2 rows each)
NSUB = H // 2                   # 32 query subs (128 queries each)
NGRP = NSUB // 4                # 8 groups of 4 subs (one PSUM bank each)
SCALE = 1.0 / np.sqrt(np.float32(CH))

# ---- const blob layout (bf16 [128, BLOBW]) ----
O_W2T = 0            # [128,128]
O_WOT = 128          # [128,256]
O_M2G = 384          # maskC2g [128,896]
O_IDENT = 1280       # [128,128]
O_ONESC = 1408       # [128,1] ones column
O_ONESR = 1410       # [1,128] ones row (row 0)
O_B1 = 1538          # [128,1] f32 (2 bf16 cols)
O_B2H = 1540         # [128,1] f32
O_BOUT = 1542        # [128,2] f32 (4 bf16 cols)
O_ONESRF = 1546      # [1,128] f32 ones row (256 bf16 cols, row 0)
O_D2R = 1802         # [1,512] bf16 (row 0)
O_ZERO = 2336        # [128,128] bf16 zeros
BLOBW = 2464


def _build_masks():
    """maskC2g: (128, 896) {0,1} bf16 = maskC | zeros(128) | maskC.
    maskC col 128*a+q is key (chunk c, pos p) vs query q of sub s=c-2+a:
    valid iff |2-2a + p//64 - q//64| <= 2 and |p%64 - q%64| <= 2."""
    m = np.zeros((128, 384), dtype=np.float32)
    for a in range(3):
        for p in range(128):
            for q in range(128):
                dy = 2 - 2 * a + p // 64 - q // 64
                if abs(dy) <= RAD and abs(p % 64 - q % 64) <= RAD:
                    m[p, 128 * a + q] = 1.0
    m2g = np.concatenate([m, np.zeros((128, 128), np.float32), m], axis=1)

    cnt = np.array([sum(1 for dx in range(-RAD, RAD + 1) if not 0 <= qx + dx < W)
                    for qx in range(W)], dtype=np.float32)
    d2row = 2.0 * 5.0 * np.concatenate([cnt, cnt])          # (128,) 2*D
    return m2g.astype(ml_dtypes.bfloat16), d2row


def _chunk_parts(c):
    """den/samp MM parts for chunk c: (s_lo, s_hi, stop) sub-ranges split
    at 4-sub PSUM bank boundaries.  Banks are pre-zeroed by one dedicated
    matmul, so every part is a plain accumulate (start=False)."""
    smin, smax = max(0, c - 2), min(NSUB - 1, c)
    parts = []
    s = smin
    while s <= smax:
        e = min(smax, (s // 4) * 4 + 3)           # clip to bank
        g = s // 4
        stop = c == min(4 * g + 5, NCHUNK - 1) and e == min(4 * g + 3, NSUB - 1)
        parts.append((s, e, stop))
        s = e + 1
    return parts


def build_nc(repeat=1, sim_safe=False):
    nc = bacc.Bacc("TRN2", target_bir_lowering=False, debug=False, num_devices=8)

    x_d = nc.declare_dram_parameter("x", [CIN, HW], BF16, isOutput=False)
    w1t_d = nc.declare_dram_parameter("W1T", [CIN, CH], BF16, isOutput=False)
    blob_d = nc.declare_dram_parameter("blob", [128, BLOBW], BF16, isOutput=False)
    out_d = nc.declare_dram_parameter("out", [CIN, HW], BF16, isOutput=True)

    with tile.TileContext(nc) as tc:
        with (
            tc.tile_pool(name="per", bufs=1) as per,
            tc.tile_pool(name="smp", bufs=8) as smp,
            tc.tile_pool(name="otp", bufs=6) as otp,
            tc.tile_pool(name="pA", bufs=2, space="PSUM") as pA,   # 2x[128,1024] f32
            tc.tile_pool(name="pB", bufs=2, space="PSUM") as pB,   # 2x[128,512] f32
            tc.tile_pool(name="pD", bufs=2, space="PSUM") as pD,   # 2x[1,512] f32
        ):
            blobw = per.tile([128, 2 * CH], BF16, tag="blobw")
            blob = per.tile([128, BLOBW], BF16, tag="blob")
            xall = per.tile([128, 2 * HW], BF16, tag="xall")
            hpad = per.tile([128, PADPOS], BF16, tag="hpad")
            hT = per.tile([128, PADPOS], BF16, tag="hT")
            attnm = per.tile([128, NCHUNK * 512], BF16, tag="attnm")
            Pg = per.tile([128, HW], BF16, tag="Pg")
            attr = per.tile([128, HW], BF16, tag="attr")

            w1t0 = blobw[:, 0:CH]
            w1t1 = blobw[:, CH:2 * CH]
            w2t = blob[:, O_W2T:O_W2T + 128]
            wot = blob[:, O_WOT:O_WOT + 256]
            maskC2g = blob[:, O_M2G:O_M2G + 896]
            maskC = blob[:, O_M2G:O_M2G + 384]
            ident = blob[:, O_IDENT:O_IDENT + 128]
            onescol = blob[:, O_ONESC:O_ONESC + 1]
            onesrow = blob[0:1, O_ONESR:O_ONESR + 128]
            onesrow_f32 = blob[0:1, O_ONESRF:O_ONESRF + 256].bitcast(F32)
            b1 = blob[:, O_B1:O_B1 + 2].bitcast(F32)
            b2h = blob[:, O_B2H:O_B2H + 2].bitcast(F32)
            bout0 = blob[:, O_BOUT:O_BOUT + 2].bitcast(F32)
            bout1 = blob[:, O_BOUT + 2:O_BOUT + 4].bitcast(F32)
            d2row = blob[0:1, O_D2R:O_D2R + 512]
            zeroblk = blob[:, O_ZERO:O_ZERO + 128]
            zerocol = blob[:, O_ZERO:O_ZERO + 1]

            # --- input DMAs: weights first, then x in 1024-col blocks,
            # split across the SP and ACT issue queues.
            # Weights and the first x block in fine-grained DMAs so the
            # first conv1 matmul can start as early as possible (each DMA
            # streams on a single ~22GB/s engine; parallelism comes from
            # issuing many).
            nc.sync.dma_start(blobw[:, 0:CH], w1t_d[0:128, :])
            nc.scalar.dma_start(blobw[:, CH:2 * CH], w1t_d[128:256, :])
            engs = [nc.sync, nc.scalar, nc.gpsimd]
            k = 0
            for half in range(2):                 # cin half
                for c0 in range(0, 512, 256):     # first 512 cols, fine
                    src = x_d[128 * half:128 * (half + 1), c0:c0 + 256]
                    engs[k % 3].dma_start(xall[:, HW * half + c0:HW * half + c0 + 256], src)
                    k += 1
            # blob in 4 pieces so ident/masks land early and no single
            # ~22GB/s ring serializes the whole 584KB
            for j, (c0, c1) in enumerate(((O_IDENT, BLOBW), (O_M2G, O_M2G + 896),
                                          (0, O_M2G))):
                engs[(k + j) % 3].dma_start(blob[:, c0:c1], blob_d[:, c0:c1])
            k += 3
            for half in range(2):
                src = x_d[128 * half:128 * (half + 1), 512:1024]
                engs[k % 3].dma_start(xall[:, HW * half + 512:HW * half + 1024], src)
                k += 1
            for u in range(1, 4):
                for half in range(2):
                    for c0 in range(1024 * u, 1024 * (u + 1), 512):
                        src = x_d[128 * half:128 * (half + 1), c0:c0 + 512]
                        engs[k % 3].dma_start(
                            xall[:, HW * half + c0:HW * half + c0 + 512], src)
                        k += 1

            # pad chunks (0 and 33) are identically zero
            nc.vector.memset(hpad[:, 0:128], 0.0)
            nc.vector.memset(hpad[:, PADPOS - 128:PADPOS], 0.0)
            nc.gpsimd.memset(hT[:, 0:128], 0.0)
            nc.gpsimd.memset(hT[:, PADPOS - 128:PADPOS], 0.0)

            for _rep in range(repeat):
                # ---- P1: conv1 + transposes + conv2, PE kept streaming.
                def emit_transp_group(u):
                    pt = pA.tile([128, 1024], BF16, tag="pa", name=f"pt{u}")
                    for k in range(8):
                        c = 8 * u + 1 + k
                        nc.tensor.transpose(pt[:, 128 * k:128 * (k + 1)],
                                            hpad[:, 128 * c:128 * (c + 1)],
                                            ident)
                    nc.vector.tensor_copy(hT[:, 128 * (8 * u + 1):128 * (8 * u + 9)],
                                          pt[:])

                def emit_conv2(b):
                    pz = pB.tile([128, 512], F32, tag="pb", name=f"pz{b}")
                    nc.tensor.matmul(pz[:], w2t,
                                     hpad[:, 128 + 512 * b:128 + 512 * (b + 1)],
                                     start=True, stop=True)
                    tg = smp.tile([128, 512], BF16, tag="tg")
                    nc.scalar.activation(tg[:], pz[:], AF.Tanh, scale=0.5, bias=b2h)
                    nc.vector.tensor_scalar(
                        out=Pg[:, 512 * b:512 * (b + 1)], in0=tg[:],
                        scalar1=0.0, scalar2=1.0, op0=ALU.max, op1=ALU.add)

                for u in range(4):
                    cvt = pA.tile([128, 1024], F32, tag="pa", name=f"cv{u}")
                    for h2 in range(2):
                        dst = cvt[:, 512 * h2:512 * (h2 + 1)]
                        cs = slice(1024 * u + 512 * h2, 1024 * u + 512 * (h2 + 1))
                        cs2 = slice(HW + cs.start, HW + cs.stop)
                        nc.tensor.matmul(dst, w1t0, xall[:, cs], start=True, stop=False)
                        nc.tensor.matmul(dst, w1t1, xall[:, cs2], start=False, stop=True)
                    nc.scalar.activation(
                        hpad[:, 128 + 1024 * u:128 + 1024 * (u + 1)],
                        cvt[:], AF.Identity, bias=b1, scale=1.0)
                    if u >= 1:
                        emit_transp_group(u - 1)
                        emit_conv2(2 * (u - 1))
                        emit_conv2(2 * (u - 1) + 1)
                emit_transp_group(3)
                emit_conv2(6)
                emit_conv2(7)

                # ---- P2: chunk pipeline: scores/exp/mask with den+samp
                # matmuls trailing two pairs behind on the PE.
                deng = {}
                sampg = {}

                def emit_score_pair(cp):
                    sc = pA.tile([128, 1024], F32, tag="pa", name=f"sc{cp}")
                    spans = []
                    for ci in range(2):
                        c = 2 * cp + ci
                        lo, hi = max(0, c - 2), min(NSUB - 1, c)
                        alo = lo - (c - 2)
                        spans.append((alo, alo + hi - lo + 1))
                        nc.tensor.matmul(
                            sc[:, 512 * ci + 128 * alo:512 * ci + 128 * (alo + hi - lo + 1)],
                            hpad[:, 128 * c:128 * (c + 1)],
                            hpad[:, 128 * (lo + 1):128 * (hi + 2)],
                            start=True, stop=True)
                    if spans == [(0, 3), (0, 3)]:
                        asl = attnm[:, 1024 * cp:1024 * cp + 896]
                        nc.scalar.activation(asl, sc[:, 0:896], AF.Exp,
                                             scale=float(SCALE))
                        nc.vector.tensor_tensor(out=asl, in0=asl,
                                                in1=maskC2g, op=ALU.mult)
                    else:
                        for ci, (a0, a1) in enumerate(spans):
                            ss = slice(512 * ci + 128 * a0, 512 * ci + 128 * a1)
                            asl = attnm[:, 1024 * cp + ss.start:1024 * cp + ss.stop]
                            nc.scalar.activation(asl, sc[:, ss], AF.Exp,
                                                 scale=float(SCALE))
                            nc.vector.tensor_tensor(
                                out=asl, in0=asl,
                                in1=maskC[:, 128 * a0:128 * a1], op=ALU.mult)

                zbs = {}

                def emit_recip_chain(g):
                    # reciprocal straight from den PSUM: two thin DVE ops +
                    # an ACT cast to bf16.  No DMAs, no GPSIMD (its Q7
                    # program swaps cost ~5-9us).  The partition broadcast
                    # happens later as one PE matmul in the convout phase.
                    z = smp.tile([1, 512], F32, tag="z", name=f"z{g}")
                    nc.vector.scalar_tensor_tensor(
                        out=z[:], in0=deng[g][0:1, :], scalar=2.0,
                        in1=d2row, op0=ALU.mult, op1=ALU.add)
                    nc.vector.reciprocal_approx_fast(z[:], z[:])
                    zb = smp.tile([1, 512], BF16, tag="zb", name=f"zb{g}")
                    nc.scalar.copy(zb[:], z[:])
                    zbs[g] = zb

                def emit_densamp_chunk(c):
                    parts = _chunk_parts(c)
                    for s, e, sp in parts:
                        g = s // 4
                        if g not in deng:
                            deng[g] = pD.tile([1, 512], F32, tag="pd", name=f"dn{g}")
                            sampg[g] = pB.tile([128, 512], F32, tag="pb", name=f"sp{g}")
                            # pre-zero both banks (start=True pends the whole
                            # 2KB zero region; the write clears all of it)
                            nc.tensor.matmul(deng[g][0:1, :], zerocol,
                                             blob[:, 0:512], start=True, stop=False)
                            nc.tensor.matmul(sampg[g][:], zeroblk,
                                             blob[:, 0:512], start=True, stop=False)
                        aa = s - (c - 2)
                        rhs = attnm[:, 512 * c + 128 * aa:512 * c + 128 * (aa + e - s + 1)]
                        nc.tensor.matmul(
                            deng[g][0:1, 128 * (s - 4 * g):128 * (e + 1 - 4 * g)],
                            onescol, rhs, start=False, stop=sp)
                    for s, e, sp in parts:
                        g = s // 4
                        nc.tensor.matmul(
                            sampg[g][:, 128 * (s - 4 * g):128 * (e + 1 - 4 * g)],
                            hT[:, 128 * c:128 * (c + 1)],
                            attnm[:, 512 * c + 128 * (s - (c - 2)):
                                  512 * c + 128 * (s - (c - 2) + e - s + 1)],
                            start=False, stop=sp)

                def emit_group_done(g):
                    gsl = slice(512 * g, 512 * (g + 1))
                    # attr = samp * Pg (unnormalized; recip applied in P3)
                    nc.vector.tensor_tensor(out=attr[:, gsl], in0=sampg[g][:],
                                            in1=Pg[:, gsl], op=ALU.mult)

                for cp in range(19):
                    if cp <= 16:
                        emit_score_pair(cp)
                    if cp >= 2:
                        for c in (2 * (cp - 2), 2 * (cp - 2) + 1):
                            emit_densamp_chunk(c)
                            if c >= 5 and (c - 5) % 4 == 0:
                                emit_recip_chain((c - 5) // 4)
                                emit_group_done((c - 5) // 4)

                # ---- P3: broadcast each group's reciprocal row with one PE
                # matmul (f32, one group ahead so the DVE normalize hides
                # behind the previous group's conv), then the output conv.
                def emit_pb(g):
                    pb = pD.tile([128, 512], F32, tag="pd", name=f"pb{g}")
                    nc.tensor.matmul(pb[:], onesrow, zbs[g][:],
                                     start=True, stop=True)
                    gsl = slice(512 * g, 512 * (g + 1))
                    nc.vector.tensor_tensor(out=attr[:, gsl], in0=attr[:, gsl],
                                            in1=pb[:], op=ALU.mult)

                emit_pb(0)
                emit_pb(1)
                for g in range(NGRP):
                    if g + 2 < NGRP:
                        emit_pb(g + 2)
                    gsl = slice(512 * g, 512 * (g + 1))
                    po0 = pB.tile([128, 512], F32, tag="pb", name=f"po0_{g}")
                    po1 = pA.tile([128, 1024], F32, tag="pa", name=f"po1_{g}")
                    nc.tensor.matmul(po0[:], wot[:, 0:128], attr[:, gsl],
                                     start=True, stop=True)
                    nc.tensor.matmul(po1[:, 0:512], wot[:, 128:256], attr[:, gsl],
                                     start=True, stop=True)
                    ot = otp.tile([128, 1024], BF16, tag="ot")
                    if g % 2 == 0:
                        nc.vector.tensor_scalar(out=ot[:, 0:512], in0=po0[:],
                                                scalar1=bout0, scalar2=None,
                                                op0=ALU.add)
                        nc.scalar.activation(ot[:, 512:1024], po1[:, 0:512],
                                             AF.Identity, bias=bout1, scale=1.0)
                    else:
                        nc.scalar.activation(ot[:, 0:512], po0[:],
                                             AF.Identity, bias=bout0, scale=1.0)
                        nc.vector.tensor_scalar(out=ot[:, 512:1024],
                                                in0=po1[:, 0:512],
                                                scalar1=bout1, scalar2=None,
                                                op0=ALU.add)
                    # split stores so each rides its own ~22GB/s DMA ring,
                    # issues round-robin over three queues; finer for the
                    # last group to shorten the drain tail
                    nsp = 2 if g == NGRP - 1 else 1
                    for oc in range(2):
                        osl = slice(512 * oc, 512 * (oc + 1))
                        for j in range(nsp):
                            w = 512 // nsp
                            qs = slice(512 * g + w * j, 512 * g + w * (j + 1))
                            ts = slice(osl.start + w * j, osl.start + w * (j + 1))
                            eng = engs[(2 * g + oc + j) % 3]
                            eng.dma_start(out_d[128 * oc:128 * (oc + 1), qs],
                                          ot[:, ts])

    return nc


def _prep_inputs(x, W1, b1, W2, b2, Wout, bout):
    m2g, d2row = _build_masks()
    bf = ml_dtypes.bfloat16

    blob = np.zeros((128, BLOBW), dtype=np.float32)
    blob_bf = blob.astype(bf)

    def put_bf(col, arr):
        arr = np.asarray(arr).astype(bf)
        blob_bf[:arr.shape[0], col:col + arr.shape[1]] = arr

    def put_f32(col, arr):
        arr = np.ascontiguousarray(np.asarray(arr, np.float32))
        v = arr.view(np.uint16).reshape(arr.shape[0], -1)
        blob_bf[:arr.shape[0], col:col + v.shape[1]] = v.view(bf)

    put_bf(O_W2T, np.ascontiguousarray(W2.T))
    put_bf(O_WOT, np.ascontiguousarray(Wout.T))
    put_bf(O_M2G, m2g)
    put_bf(O_IDENT, np.eye(128, dtype=np.float32))
    put_bf(O_ONESC, np.ones((128, 1), np.float32))
    put_bf(O_ONESR, np.ones((1, 128), np.float32))
    put_f32(O_B1, np.asarray(b1, np.float32).reshape(CH, 1))
    put_f32(O_B2H, (0.5 * np.asarray(b2, np.float32)).reshape(CH, 1))
    put_f32(O_BOUT, np.ascontiguousarray(
        np.asarray(bout, np.float32).reshape(2, CH).T))
    put_f32(O_ONESRF, np.ones((1, 128), np.float32))
    put_bf(O_D2R, np.tile(d2row[None, :], (1, 4)).reshape(1, 512))

    common = {
        "W1T": np.ascontiguousarray(W1.T).astype(bf),
        "blob": blob_bf,
    }
    in_maps = []
    for i in range(N):
        m = dict(common)
        m["x"] = np.ascontiguousarray(
            np.asarray(x[i], np.float32).reshape(CIN, HW)).astype(bf)
        in_maps.append(m)
    return in_maps


_CACHED = {}


def kernel(x, W1, b1, W2, b2, Wout, bout):
    if "nc" not in _CACHED:
        nc = build_nc()
        nc.finalize()
        _CACHED["nc"] = nc
    nc = _CACHED["nc"]
    in_maps = _prep_inputs(x, W1, b1, W2, b2, Wout, bout)
    res = run_bass_kernel_spmd(nc, in_maps, core_ids=list(range(N)))
    out = np.stack([np.asarray(res.results[i]["out"], dtype=np.float32)
                    .reshape(CIN, H, W) for i in range(N)])
    return out

